# revision 17
# baseline (speedup 1.0000x reference)
"""GRU kernel for Trainium2 (Bass/Tile), 8-core batch-parallel.

Problem: x [T=2048, B=64, D=256] fp32, h0 [64, 512], Wz/Wr/Wh [768, 512],
bz/br/bh [512]. Returns hidden history [T, 64, 512] fp32.

Strategy:
  - Data-parallel over batch: core c handles batch rows c*8:(c+1)*8.
  - All on-device activations live in "transposed" layout: hidden dim on the
    128 partitions (4 column-groups of 8 batch cols), batch on the free dim.
  - Per timestep, the 3 gate matmuls run with the weight tile stationary
    (lhsT = W[k,m] 128x128 bf16) and hT streaming as an 8-column rhs.
  - The x-dependent part of all gates (x_t @ W[:D] + b) is batched over a
    64-step time tile as dense N=512 matmuls, cast to bf16 in SBUF, and
    injected into each step's PSUM accumulation via an identity matmul.
  - Host pre-transposes x to xT bf16 and post-transposes the output, so the
    device never transposes anything.
"""

import os
import sys

for _p in ("/opt/trn_rl_repo", os.path.expanduser("~/.axon_site/_ro/trn_rl_repo")):
    if os.path.isdir(_p) and _p not in sys.path:
        sys.path.insert(0, _p)

import numpy as np
import ml_dtypes

T, B, D, H = 2048, 64, 256, 512
NCORES = 8
BC = B // NCORES              # 8 batch rows per core
G = H // 128                  # 4 column-groups of the hidden dim
KH = H // 128                 # 4 contraction chunks for the h-part
KX = D // 128                 # 2 contraction chunks for the x-part
TT = 64                       # timesteps per loop iteration
BF16 = ml_dtypes.bfloat16


def _prep_w(w):
    # W [768, 512] -> [128, 6*512] bf16; col = k*512 + m*128 + j holds W[k*128+p, m*128+j]
    return np.ascontiguousarray(
        w.reshape(6, 128, 4, 128).transpose(1, 0, 2, 3).reshape(128, 3072)
    ).astype(BF16)


def _build_program(n_tiles):
    import concourse.bass as bass
    import concourse.tile as tile
    from concourse import bacc, mybir

    fp32 = mybir.dt.float32
    bf16 = mybir.dt.bfloat16
    Tn = n_tiles * TT

    nc = bacc.Bacc(
        "TRN2",
        target_bir_lowering=False,
        debug=False,
        enable_asserts=False,
        num_devices=NCORES,
    )

    assert n_tiles % 2 == 0, "loop body processes two tiles"
    # one tile of zero padding at the end for the final (dead) prefetch
    xT_d = nc.dram_tensor("xT", [D, (Tn + TT) * BC], bf16, kind="ExternalInput")
    h0T_d = nc.dram_tensor("h0T", [128, G * BC], fp32, kind="ExternalInput")
    w_d = {
        g: nc.dram_tensor(f"W{g}", [128, 3072], bf16, kind="ExternalInput")
        for g in "zrh"
    }
    b_d = nc.dram_tensor("bT", [128, 12], fp32, kind="ExternalInput")
    id_d = nc.dram_tensor("ident", [128, 128], bf16, kind="ExternalInput")
    hist_d = nc.dram_tensor("histT", [128, Tn * G * BC], fp32, kind="ExternalOutput")

    from contextlib import ExitStack

    with tile.TileContext(nc) as tc, ExitStack() as ctx:
        persist = ctx.enter_context(tc.tile_pool(name="persist", bufs=1))
        wsb = {
            g: persist.tile([128, 3072], bf16, tag=f"W{g}", name=f"W{g}sb")
            for g in "zrh"
        }
        bsb = persist.tile([128, 12], fp32, tag="bT")
        ident = persist.tile([128, 128], bf16, tag="ident")
        h_cf = persist.tile([128, G * BC], fp32, tag="h_carry_f")   # carry h fp32
        h_cb = persist.tile([128, G * BC], bf16, tag="h_carry_b")   # carry h bf16

        for g in "zrh":
            nc.sync.dma_start(wsb[g][:], w_d[g].ap()[:])
        nc.sync.dma_start(bsb[:], b_d.ap()[:])
        nc.sync.dma_start(ident[:], id_d.ap()[:])
        nc.sync.dma_start(h_cf[:], h0T_d.ap()[:])
        nc.vector.tensor_copy(h_cb[:], h_cf[:])

        xt_pool = ctx.enter_context(tc.tile_pool(name="xT", bufs=3))
        xg_pool = ctx.enter_context(tc.tile_pool(name="xg", bufs=2))
        hist_pool = ctx.enter_context(tc.tile_pool(name="hist", bufs=2))
        sm_pool = ctx.enter_context(tc.tile_pool(name="small", bufs=3))
        ps_r = ctx.enter_context(tc.tile_pool(name="ps_r", bufs=2, space="PSUM"))
        ps_zr = ctx.enter_context(tc.tile_pool(name="ps_zr", bufs=2, space="PSUM"))
        ps_c = ctx.enter_context(tc.tile_pool(name="ps_c", bufs=2, space="PSUM"))
        ps_xg = ctx.enter_context(tc.tile_pool(name="ps_xg", bufs=2, space="PSUM"))

        xga = persist.tile([128, 12 * TT * BC], bf16, tag="xga")
        xgb = persist.tile([128, 12 * TT * BC], bf16, tag="xgb")

        def wtile(g, k, m):
            # lhsT tile for gate g, contraction chunk k (0,1=x-part, 2..5=h-part),
            # output chunk m
            return wsb[g][:, k * 512 + m * 128 : k * 512 + (m + 1) * 128]

        def emit_xfill_load(xt_col_start):
            """DMA the xT slice for one future tile. xt_col_start: col offset."""
            xts = []
            for k in range(KX):
                xt = xt_pool.tile([128, TT * BC], bf16, tag=f"xT{k}", name=f"xt{k}")
                nc.sync.dma_start(
                    xt[:],
                    xT_d.ap()[
                        k * 128 : (k + 1) * 128,
                        bass.DynSlice(xt_col_start, TT * BC)
                        if not isinstance(xt_col_start, int)
                        else slice(xt_col_start, xt_col_start + TT * BC),
                    ],
                )
                xts.append(xt)
            return xts

        def emit_xfill_mm(xg_t, xts, m12):
            """One gate-chunk of the batched x-part into xg_t (+bias, bf16)."""
            gate = "zrh"[m12 // 4]
            m = m12 % 4
            pxg = ps_xg.tile([128, TT * BC], fp32, tag="ps_xg", name="pxg")
            for k in range(KX):
                nc.tensor.matmul(
                    pxg[:],
                    wtile(gate, k, m),
                    xts[k][:],
                    start=(k == 0),
                    stop=(k == KX - 1),
                )
            nc.scalar.activation(
                xg_t[:, m12 * TT * BC : (m12 + 1) * TT * BC],
                pxg[:],
                mybir.ActivationFunctionType.Identity,
                bias=bsb[:, m12 : m12 + 1],
            )

        def recurrence(xg_use, xg_fill, fill_col_start, hist_col_start):
            """TT steps using xg_use; interleave x-part fill of xg_fill."""
            xg3 = xg_use[:].rearrange("p (m s) -> p m s", m=12)
            hist = hist_pool.tile([128, TT * G * BC], fp32, tag="hist", name="hist")
            prev_hb = None
            fill_xts = None
            for s in range(TT):
                h_prev_f = h_cf[:] if s == 0 else hist[:, (s - 1) * 32 : s * 32]
                h_prev_b = h_cb[:] if s == 0 else prev_hb[:]

                # padded to a full 2KB PSUM bank each, so pool slots never
                # share a bank (bank-overlap tracking would serialize ACT
                # reads of step s behind PE writes of step s+1)
                pr = ps_r.tile(
                    [128, 32], fp32, tag="ps_r", name="pr", padded_shape=[128, 512]
                )
                pz = ps_zr.tile(
                    [128, 32], fp32, tag="ps_z", name="pz", padded_shape=[128, 512]
                )
                pc = ps_c.tile(
                    [128, 32], fp32, tag="ps_c", name="pc", padded_shape=[128, 512]
                )
                # inject the precomputed x-parts (+bias)
                nc.tensor.matmul(
                    pr[:], ident[:], xg3[:, 4:8, s * BC : (s + 1) * BC],
                    start=True, stop=False, skip_group_check=True,
                )
                nc.tensor.matmul(
                    pz[:], ident[:], xg3[:, 0:4, s * BC : (s + 1) * BC],
                    start=True, stop=False, skip_group_check=True,
                )
                nc.tensor.matmul(
                    pc[:], ident[:], xg3[:, 8:12, s * BC : (s + 1) * BC],
                    start=True, stop=False, skip_group_check=True,
                )
                # r gate (own PSUM bank so sigma_r fires as soon as r MMs end)
                r_sb = sm_pool.tile([128, 32], fp32, tag="r_sb", name="r_sb")
                rh_b = sm_pool.tile([128, 32], bf16, tag="rh_b", name="rh_b")
                for k in range(KH):
                    for m in range(4):
                        nc.tensor.matmul(
                            pr[:, m * 8 : (m + 1) * 8],
                            wtile("r", 2 + k, m),
                            h_prev_b[:, k * 8 : (k + 1) * 8],
                            start=False, stop=(k == KH - 1 and m == 3),
                            skip_group_check=True,
                        )
                nc.scalar.activation(
                    r_sb[:], pr[:], mybir.ActivationFunctionType.Sigmoid
                )
                nc.vector.tensor_mul(rh_b[:], r_sb[:], h_prev_f[:])
                # z gate
                z_sb = sm_pool.tile([128, 32], fp32, tag="z_sb", name="z_sb")
                w_sb = sm_pool.tile([128, 32], fp32, tag="w_sb", name="w_sb")
                v_sb = sm_pool.tile([128, 32], fp32, tag="v_sb", name="v_sb")
                for k in range(KH):
                    for m in range(4):
                        nc.tensor.matmul(
                            pz[:, m * 8 : (m + 1) * 8],
                            wtile("z", 2 + k, m),
                            h_prev_b[:, k * 8 : (k + 1) * 8],
                            start=False, stop=(k == KH - 1 and m == 3),
                            skip_group_check=True,
                        )
                nc.scalar.activation(
                    z_sb[:], pz[:], mybir.ActivationFunctionType.Sigmoid
                )
                # w = 1 - z  (sigmoid of negated preact); v = w * h_prev
                nc.scalar.activation(
                    w_sb[:], pz[:],
                    mybir.ActivationFunctionType.Sigmoid, scale=-1.0,
                )
                nc.vector.tensor_mul(v_sb[:], w_sb[:], h_prev_f[:])
                # candidate
                for k in range(KH):
                    for m in range(4):
                        nc.tensor.matmul(
                            pc[:, m * 8 : (m + 1) * 8],
                            wtile("h", 2 + k, m),
                            rh_b[:, k * 8 : (k + 1) * 8],
                            start=False, stop=(k == KH - 1 and m == 3),
                            skip_group_check=True,
                        )
                c_sb = sm_pool.tile([128, 32], fp32, tag="c_sb", name="c_sb")
                nc.scalar.activation(
                    c_sb[:], pc[:], mybir.ActivationFunctionType.Tanh
                )
                # h_new = z*c + (1-z)*h; bf16 copy chunked so next step's first
                # r-matmul group starts as soon as its k-chunk is ready
                u_sb = sm_pool.tile([128, 32], fp32, tag="u_sb", name="u_sb")
                hb = (
                    h_cb
                    if s == TT - 1
                    else sm_pool.tile([128, 32], bf16, tag="h_b", name="h_b")
                )
                for kk in range(0, 32, 16):
                    nc.vector.tensor_mul(
                        u_sb[:, kk : kk + 16], z_sb[:, kk : kk + 16],
                        c_sb[:, kk : kk + 16],
                    )
                    nc.vector.tensor_add(
                        hb[:, kk : kk + 16], u_sb[:, kk : kk + 16],
                        v_sb[:, kk : kk + 16],
                    )
                nc.gpsimd.tensor_add(
                    hist[:, s * 32 : (s + 1) * 32], u_sb[:], v_sb[:]
                )
                if s == TT - 1:
                    nc.vector.tensor_add(h_cf[:], u_sb[:], v_sb[:])
                prev_hb = hb

                # interleave the next tile's x-part work into PE/ACT gaps
                # (emitted at end-of-step so the ACT cast queues after tanh)
                if xg_fill is not None:
                    if s == 0:
                        fill_xts = emit_xfill_load(fill_col_start)
                    if s % 5 == 1 and s // 5 < 12:
                        emit_xfill_mm(xg_fill, fill_xts, s // 5)

            nc.sync.dma_start(
                hist_d.ap()[
                    :,
                    bass.DynSlice(hist_col_start, TT * G * BC)
                    if not isinstance(hist_col_start, int)
                    else slice(hist_col_start, hist_col_start + TT * G * BC),
                ],
                hist[:],
            )

        # prologue: fill xga for tile 0
        xts0 = emit_xfill_load(0)
        for m12 in range(12):
            emit_xfill_mm(xga, xts0, m12)

        CPB = TT * BC  # xT cols per tile
        HPB = TT * G * BC  # hist cols per tile
        with tc.For_i(
            0, n_tiles // 2, 1,
            hint_engines=tuple(mybir.ALL_ENGINES),
        ) as i:
            # sub-tile 2i: consume xga, fill xgb from tile 2i+1
            recurrence(xga, xgb, i * (2 * CPB) + CPB, i * (2 * HPB))
            # sub-tile 2i+1: consume xgb, fill xga from tile 2i+2
            recurrence(xgb, xga, i * (2 * CPB) + 2 * CPB, i * (2 * HPB) + HPB)

    nc.compile()
    return nc


def _run(inputs, n_tiles=T // TT, trace=False):
    from concourse.bass_utils import run_bass_kernel_spmd

    x = np.asarray(inputs["x"], dtype=np.float32)
    h0 = np.asarray(inputs["h0"], dtype=np.float32)
    Tn = n_tiles * TT
    x = x[:Tn]

    ws = {g: _prep_w(np.asarray(inputs[f"W{g}"], dtype=np.float32)) for g in "zrh"}
    bT = np.ascontiguousarray(
        np.stack(
            [np.asarray(inputs[f"b{g}"], dtype=np.float32).reshape(4, 128).T for g in "zrh"],
            axis=1,
        ).reshape(128, 12)
    )
    ident = np.eye(128, dtype=np.float32).astype(BF16)
    xT_all = x.astype(BF16).transpose(2, 0, 1)  # [D, Tn, B]

    in_maps = []
    for c in range(NCORES):
        sl = slice(c * BC, (c + 1) * BC)
        xT = np.zeros((D, (Tn + TT) * BC), dtype=BF16)
        xT[:, : Tn * BC] = xT_all[:, :, sl].reshape(D, Tn * BC)
        h0T = np.ascontiguousarray(
            h0[sl].reshape(BC, G, 128).transpose(2, 1, 0).reshape(128, G * BC)
        )
        in_maps.append(
            {
                "xT": xT,
                "h0T": h0T,
                "Wz": ws["z"], "Wr": ws["r"], "Wh": ws["h"],
                "bT": bT,
                "ident": ident,
            }
        )

    nc = _build_program(n_tiles)
    res = run_bass_kernel_spmd(nc, in_maps, core_ids=list(range(NCORES)), trace=trace)

    out = np.empty((Tn, B, H), dtype=np.float32)
    for c in range(NCORES):
        histT = res.results[c]["histT"]  # [128, Tn*G*BC]
        out[:, c * BC : (c + 1) * BC, :] = (
            histT.reshape(128, Tn, G, BC).transpose(1, 3, 2, 0).reshape(Tn, BC, H)
        )
    return out, res


def kernel(**inputs):
    out, _ = _run(inputs)
    return out
